# revision 64
# baseline (speedup 1.0000x reference)
"""Trainium2 Bass kernel for nn_Attention_block (retrieval_knn).

Reference (per sample b, match A in {Q_flo, K_dep}, V = V_rgb):
  T[i,j] = <A[:,i], V[:,j]>          [4096, 4096] score matrix
  S[j] = max_i T ; idx[j] = argmax_i T
  C = conv1x1([V; A[:, idx]]) * S    (conv1: 128->64)
  fused = [C_v, C_k, V]              (192 ch)
  y = relu(BN(conv3x3(fused)))       (conv2: 192->64, pad 1)

Sharding: 8 cores = 4 samples x 2 W-halves (pure data parallel; each core
takes a 1-row halo each side of its half for the 3x3 conv and computes its
2176 j-columns against the full 4096-long i axis).

Device-side structure per core (252.7us cost model, 1.41x over the
357.1us fp32 baseline; DVE/ACT-bound):
  - G-trick: gather commutes with conv1's TA half:
      conv1([V;TA]) + b1 = W1v@V + (W1t@A + b1)[:, idx]
    G' = W1t@A + b1 is computed once per match, transposed into DRAM
    [4096, 64], and argmax rows are fetched by indirect-DMA gather.
  - Scores must be fp32-exact: smallest top-2 score gap here is ~1.5e-4 and
    the reference argmax is f32.  Plain fp32 matmuls cost 4 PE cycles/row
    and fp32r is TF32 (10-bit - flips argmaxes), so the spine uses an exact
    bf16 split computed on the host: A = A1 + A2, V = V1 + V2 (hi/lo bf16
    pairs); T = V1'A1 + (V1'A2 + V2'A1), where the two cross terms fold
    into ONE K=128 matmul by stacking [V1;V2] against [A2;A1] (PE cost is
    per output row, independent of K) - 2 bf16 matmuls/chunk instead of 1
    fp32 at 4 cyc/row.  |T~ - T| ~ 3e-5, 5x below the min gap.
  - Argmax spine, m-major per 128-j tile: chunk matmuls accumulate into
    2-bank PSUM pair tiles; ACT evacuates [128,1024] pairs into an SBUF row
    T_sb [128,4096].  The max+index extraction alternates two formulations
    to balance DVE vs ACT (gpsimd cannot run TensorScalarPtr, so Pool only
    gets gathers/iota/copies):
      * DVE-style (22 tiles): 2x-mode max pass (tensor_scalar accum -> S)
        + 1x is_equal*iota pass (scalar_tensor_tensor accum -> idx).
      * ACT-style (12 tiles): DVE prefix-max scan (tensor_tensor_scan
        max/bypass, in place), S = P[:,-1] (Pool copies), then
        idx = -sum(sign(P - S)) as ONE ACT Sign activation with
        per-partition bias -S and the sum accumulator.  Exact,
        first-occurrence tie semantics.  (Emitting this ACT work deferred
        breaks real-HW correctness - keep it inline.)
  - Software-pipelined emission: group g's s_group/conv1 are emitted after
    group g+1's spine so gathers/S-broadcasts are a full group stale and
    never head-block the in-order PE/Pool queues; gathers are deferred 2
    match-tiles; G' work is interleaved into spine group 0; conv2 runs as
    8-row quarters, two at g==4 and two after the last group.
  - conv1 / conv2 / G' matmuls run in bf16 (value paths; ~2.9e-3 rel err,
    well under the 2e-2 gate).  The G'->transpose->gather->conv1-TA chain
    stays fp32 (transposes must dtype-match their data, and a 16-bit
    transpose output cannot accumulate into an fp32 PSUM group).
  - PSUM (8 banks): spine 2x[128,1024] pairs (4) + conv1 cva/cvb (2) +
    conv2/transpose-staging c2a/c2b (2).  Sharing one bank between two
    64-partition accumulation groups passes the simulator but breaks on
    hardware (start=True zeroes the whole region) - keep halves in
    separate banks.
"""

import numpy as np
import ml_dtypes

import concourse.bass as bass
import concourse.bacc as bacc
import concourse.mybir as mybir
from concourse.tile import TileContext
from concourse import bass_utils
from concourse.masks import make_identity

F32 = mybir.dt.float32
BF16 = mybir.dt.bfloat16
I32 = mybir.dt.int32
AF = mybir.ActivationFunctionType
OP = mybir.AluOpType

B, C, W, H = 4, 64, 64, 64
HW = W * H                     # 4096
BN_EPS = 1e-5
N_CORES = 8
WROWS = W // 2 + 2             # 34 window rows (half + 1-row halo each side)
JW = WROWS * H                 # 2176 j-columns per core
JT = JW // 128                 # 17 j-tiles
NCH = HW // 512                # 8 i-chunks
OUT_ROWS = W // 2              # 32 interior rows per core
YPAD = H + 2                   # 66 padded y positions in fused layout

NEG = -3.0e38


def _build_nc():
    nc = bacc.Bacc("TRN2", target_bir_lowering=False)

    # Exact bf16 split pairs (X = X1 + X2 with X2 itself bf16-exact to
    # ~2^-17 rel).  T = V1'A1 + (V1'A2 + V2'A1): the two cross terms are
    # computed as ONE K=128 matmul by stacking [V1;V2] against [A2;A1] -
    # PE cost is per output row, independent of K.
    #   ahi: rows 0:64 = A1_q, rows 64:128 = A1_k
    #   axq/axk: rows 0:64 = A2_m, rows 64:128 = A1_m  (match m)
    #   vhi: V1 duplicated on both row halves; vx: rows 0:64 V1, 64:128 V2
    ahi = nc.dram_tensor("ahi", [128, HW], BF16, kind="ExternalInput")
    axq = nc.dram_tensor("axq", [128, HW], BF16, kind="ExternalInput")
    axk = nc.dram_tensor("axk", [128, HW], BF16, kind="ExternalInput")
    vhi = nc.dram_tensor("vhi", [128, JW], BF16, kind="ExternalInput")
    vx = nc.dram_tensor("vx", [128, JW], BF16, kind="ExternalInput")
    vwin = nc.dram_tensor("vwin", [C, JW], BF16, kind="ExternalInput")
    w1vt = nc.dram_tensor("w1vt", [C, C], BF16, kind="ExternalInput")
    w1tt = nc.dram_tensor("w1tt", [128, C], BF16, kind="ExternalInput")
    b1d = nc.dram_tensor("b1d", [C, 1], F32, kind="ExternalInput")
    w2ad = nc.dram_tensor("w2ad", [128, 9 * C], BF16, kind="ExternalInput")
    w2bd = nc.dram_tensor("w2bd", [C, 9 * C], BF16, kind="ExternalInput")
    bnad = nc.dram_tensor("bnad", [128, 1], F32, kind="ExternalInput")
    bnbd = nc.dram_tensor("bnbd", [128, 1], F32, kind="ExternalInput")
    yout = nc.dram_tensor("y", [C, OUT_ROWS * H], F32, kind="ExternalOutput")

    if True:
      with TileContext(nc) as tc:
        with tc.tile_pool(name="persist", bufs=1) as pp:
            ahi_t = pp.tile([128, HW], BF16)
            axq_t = pp.tile([128, HW], BF16)
            axk_t = pp.tile([128, HW], BF16)
            vhi_t = pp.tile([128, JW], BF16)
            vx_t = pp.tile([128, JW], BF16)
            w1vt_t = pp.tile([C, C], BF16)
            w1tt_t = pp.tile([128, C], BF16)
            b1_t = pp.tile([C, 1], F32)
            w2a_t = pp.tile([128, 9 * C], BF16)
            w2b_t = pp.tile([C, 9 * C], BF16)
            bna_t = pp.tile([128, 1], F32)
            bnb_t = pp.tile([128, 1], F32)
            iota_t = pp.tile([128, HW], F32)
            ident = pp.tile([128, 128], F32)
            gtile = [pp.tile([128, JT * C], F32, tag="gtq", name="gtq_t"),
                     pp.tile([128, JT * C], F32, tag="gtk", name="gtk_t")]
            s_all = [pp.tile([128, JT], F32, tag="sq", name="sq_t"),
                     pp.tile([128, JT], F32, tag="sk", name="sk_t")]
            idx_all = [pp.tile([128, JT], I32, tag="idxq", name="idxq_t"),
                       pp.tile([128, JT], I32, tag="idxk", name="idxk_t")]
            s_bc = pp.tile([128, JW], F32)     # rows 0:64 Sq, 64:128 Sk
            fused_a = pp.tile([128, WROWS * YPAD], BF16)  # C_v / C_k
            fused_b = pp.tile([C, WROWS * YPAD], BF16)    # V, y-padded
            out_sb = pp.tile([128, OUT_ROWS * H // 2], F32)

            # load order: the first spine tile needs only ahi/vhi/axq
            nc.sync.dma_start(out=ahi_t[:], in_=ahi[:])
            nc.sync.dma_start(out=vhi_t[:], in_=vhi[:])
            nc.sync.dma_start(out=axq_t[:], in_=axq[:])
            nc.sync.dma_start(out=vx_t[:], in_=vx[:])
            nc.sync.dma_start(out=axk_t[:], in_=axk[:])
            nc.sync.dma_start(out=w1vt_t[:], in_=w1vt[:])
            nc.sync.dma_start(out=w1tt_t[:], in_=w1tt[:])
            nc.sync.dma_start(out=b1_t[:], in_=b1d[:])
            nc.sync.dma_start(out=w2a_t[:], in_=w2ad[:])
            nc.sync.dma_start(out=w2b_t[:], in_=w2bd[:])
            nc.sync.dma_start(out=bna_t[:], in_=bnad[:])
            nc.sync.dma_start(out=bnb_t[:], in_=bnbd[:])
            nc.gpsimd.iota(iota_t[:], pattern=[[1, HW]], base=0,
                           channel_multiplier=0,
                           allow_small_or_imprecise_dtypes=True)
            make_identity(nc, ident[:])

            fb3 = fused_b[:].rearrange("c (x y) -> c x y", y=YPAD)
            nc.gpsimd.memset(fused_b[:], 0.0)
            nc.sync.dma_start(
                out=fb3[:, :, 1:H + 1],
                in_=vwin[:].rearrange("c (x y) -> c x y", y=H))
            nc.gpsimd.memset(gtile[0][:], 0.0)
            nc.gpsimd.memset(gtile[1][:], 0.0)

            with tc.tile_pool(name="gdram", bufs=1, space="DRAM") as gdr:
                gt_dram = [gdr.tile([HW, C], F32, tag="gtdq", name="gtdq_t"),
                           gdr.tile([HW, C], F32, tag="gtdk", name="gtdk_t")]

                # ---- Phases 3-6 interleaved ----
                fa3 = fused_a[:].rearrange("c (x y) -> c x y", y=YPAD)
                nc.gpsimd.memset(fa3[:, :, 0:1], 0.0)
                nc.gpsimd.memset(fa3[:, :, YPAD - 1:YPAD], 0.0)
                with tc.tile_pool(name="sp_ps", bufs=1, space="PSUM") as sps, \
                     tc.tile_pool(name="sp_sb", bufs=2) as ssb, \
                     tc.tile_pool(name="sp_sm", bufs=4) as ssm, \
                     tc.tile_pool(name="cv_ps", bufs=1, space="PSUM") as cvp, \
                     tc.tile_pool(name="s4_sb", bufs=2) as s4, \
                     tc.tile_pool(name="s4_dram", bufs=1, space="DRAM") as d4:

                    # ---- Phase 1+2: G' = W1t @ A1 + b1; transpose to DRAM.
                    # Emitted as closures interleaved into spine group 0 so
                    # the startup isn't serial (spine doesn't depend on G').
                    g_sbs = [pp.tile([C, HW], F32, tag="gsbq", name="gsbq"),
                             pp.tile([C, HW], F32, tag="gsbk", name="gsbk")]

                    def g_chunk(c8):
                        pms = [cvp.tile([C, 512], F32, tag="cva", name="gmq"),
                               cvp.tile([C, 512], F32, tag="cvb", name="gmk")]
                        for m in range(2):
                            ro = m * C
                            nc.tensor.matmul(
                                pms[m][:], w1tt_t[ro:ro + C, :],
                                ahi_t[ro:ro + C, c8 * 512:(c8 + 1) * 512],
                                start=True, stop=True,
                                tile_position=(ro, 0))
                        for m in range(2):
                            nc.scalar.activation(
                                g_sbs[m][:, c8 * 512:(c8 + 1) * 512],
                                pms[m][:],
                                AF.Identity, bias=b1_t[:, 0:1], scale=1.0)

                    def g_transpose(m, grp):
                        g_sb = g_sbs[m]
                        pst = cvp.tile([128, 512], F32, tag="c2a",
                                       name="gtr")
                        stg = pp.tile([128, 512], F32, tag="stg")
                        for t in range(8):
                            blk = grp * 8 + t
                            nc.tensor.matmul(
                                pst[:, t * C:(t + 1) * C],
                                g_sb[:, blk * 128:(blk + 1) * 128],
                                ident[0:C, 0:C], is_transpose=True,
                                start=True, stop=True)
                        nc.scalar.copy(stg[:], pst[:])
                        nc.sync.dma_start(
                            out=gt_dram[m][:]
                            .rearrange("(g p) c -> p g c", p=128)
                            [:, grp * 8:(grp + 1) * 8, :],
                            in_=stg[:].rearrange("p (g c) -> p g c", c=C))

                    g_work = [lambda c8=c8: g_chunk(c8) for c8 in range(NCH)]
                    g_work += [lambda m=m, grp=grp: g_transpose(m, grp)
                               for m in range(2) for grp in range(4)]

                    pend_gather = []
                    pend_count = []
                    neg1_t = pp.tile([128, 1], F32, tag="neg1",
                                     name="neg1")
                    nc.gpsimd.memset(neg1_t[:], -1.0)

                    def spine_mjt(m, jt, use_act):
                        # One match x one 128-j tile: 8 i-chunk matmul
                        # triples into 2-bank PSUM pair tiles, ACT evacuates
                        # [128,1024] pairs into T_sb, DVE does the 2x max
                        # pass, and the 1x is_equal*iota index pass runs on
                        # DVE or Pool (gpsimd) per the balance schedule.
                        tsb = ssb.tile([128, HW], F32, tag=f"tsb{m}",
                                       name=f"tsb{m}")
                        ro = m * C
                        js = slice(jt * 128, (jt + 1) * 128)
                        ax_t = axq_t if m == 0 else axk_t
                        for pr in range(4):
                            ps = sps.tile([128, 1024], F32,
                                          tag=f"sp{pr % 2}", name="sp")
                            for h in range(2):
                                ch = pr * 2 + h
                                cs = slice(ch * 512, (ch + 1) * 512)
                                o = slice(h * 512, (h + 1) * 512)
                                nc.tensor.matmul(
                                    ps[:, o], vhi_t[ro:ro + C, js],
                                    ahi_t[ro:ro + C, cs],
                                    start=True, stop=False,
                                    tile_position=(ro, 0))
                                nc.tensor.matmul(
                                    ps[:, o], vx_t[:, js],
                                    ax_t[:, cs],
                                    start=False, stop=True)
                            nc.scalar.copy(
                                tsb[:, pr * 1024:(pr + 1) * 1024], ps[:])
                        S = s_all[m][:, jt:jt + 1]
                        if use_act:
                            # Offload the index pass to ScalarE: P =
                            # prefix-max(T) (DVE scan, in place), then
                            # idx = -sum(sign(P - S)) via one ACT Sign
                            # activation with per-partition bias -S and the
                            # sum accumulator.  Exact, first-occurrence.
                            # The ACT part is emitted one match-tile later
                            # (via pend_count) so ACT's in-order queue never
                            # waits on the scan.
                            nc.vector.tensor_tensor_scan(
                                out=tsb[:], data0=tsb[:], data1=tsb[:],
                                initial=NEG, op0=OP.max, op1=OP.bypass)
                            sneg = ssm.tile([128, 1], F32, tag="sneg",
                                            name="sneg")
                            nia = ssm.tile([128, 1], F32, tag="nia",
                                           name="nia")
                            nc.gpsimd.tensor_copy(
                                S, tsb[:, HW - 1:HW])
                            nc.gpsimd.tensor_tensor(
                                out=sneg[:], in0=tsb[:, HW - 1:HW],
                                in1=neg1_t[:], op=OP.mult)

                            def count_fire(m=m, jt=jt, tsb=tsb, sneg=sneg,
                                           nia=nia):
                                nc.scalar.activation(
                                    tsb[:], tsb[:], AF.Sign,
                                    bias=sneg[:, 0:1], scale=1.0,
                                    accum_out=nia[:])
                                nc.scalar.activation(
                                    idx_all[m][:, jt:jt + 1], nia[:],
                                    AF.Copy, bias=0.0, scale=-1.0)
                            count_fire()
                        else:
                            nc.vector.tensor_scalar(
                                out=tsb[:], in0=tsb[:],
                                scalar1=NEG, scalar2=NEG,
                                op0=OP.max, op1=OP.max, accum_out=S)
                            ist = ssm.tile([128, 1], F32, tag="ist",
                                           name="ist")
                            nc.vector.scalar_tensor_tensor(
                                out=tsb[:], in0=tsb[:], scalar=S,
                                in1=iota_t[:], op0=OP.is_equal,
                                op1=OP.mult, accum_out=ist[:])
                            nc.vector.tensor_copy(
                                idx_all[m][:, jt:jt + 1], ist[:])

                        def fire(m=m, jt=jt):
                            nc.gpsimd.indirect_dma_start(
                                out=gtile[m][:, jt * C:(jt + 1) * C],
                                out_offset=None,
                                in_=gt_dram[m][:],
                                in_offset=bass.IndirectOffsetOnAxis(
                                    ap=idx_all[m][:, jt:jt + 1], axis=0),
                                bounds_check=HW - 1, oob_is_err=False)
                        pend_gather.append(fire)

                    s_dram = [d4.tile([JW], F32, tag="sdq", name="sdq"),
                              d4.tile([JW], F32, tag="sdk", name="sdk")]

                    def s_group(g, jts):
                        n0 = jts[0] * 128
                        n1 = (jts[-1] + 1) * 128
                        nt = len(jts)
                        for m in range(2):
                            pst = cvp.tile([nt, 128], F32, tag="c2a",
                                           name="pst")
                            nc.tensor.matmul(
                                pst[:], s_all[m][:, jts[0]:jts[-1] + 1],
                                ident[:], is_transpose=True,
                                start=True, stop=True)
                            stg = s4.tile([JT, 128], F32, tag="stg4",
                                          name="stg4")
                            nc.scalar.copy(stg[0:nt, :], pst[:])
                            nc.sync.dma_start(
                                out=s_dram[m][n0:n1]
                                .rearrange("(t p) -> t p", p=128),
                                in_=stg[0:nt, :])
                            nc.sync.dma_start(
                                out=s_bc[m * C:(m + 1) * C, n0:n1],
                                in_=s_dram[m][None, n0:n1]
                                .to_broadcast((C, n1 - n0)))

                    def conv1_chunk(cn):
                        jts = list(range(4 * cn, min(4 * cn + 4, JT)))
                        n0 = cn * 512
                        n1 = min(n0 + 512, JW)
                        psm = [cvp.tile([128, 512], F32, tag="cva",
                                        name="cva"),
                               cvp.tile([128, 512], F32, tag="cvb",
                                        name="cvb")]
                        for m in range(2):
                            nc.tensor.matmul(
                                psm[m][m * C:(m + 1) * C, 0:n1 - n0],
                                w1vt_t[:], vhi_t[0:C, n0:n1],
                                start=True, stop=False,
                                tile_position=(0, m * C))
                        for m in range(2):
                            for i, jt in enumerate(jts):
                                if m == 0:
                                    nc.tensor.matmul(
                                        psm[m][0:C, i * 128:(i + 1) * 128],
                                        gtile[m][:, jt * C:(jt + 1) * C],
                                        ident[:], is_transpose=True,
                                        start=False, stop=(jt == jts[-1]))
                                else:
                                    nc.tensor.matmul(
                                        psm[m][C:128,
                                               i * 128:(i + 1) * 128],
                                        gtile[m][:, jt * C:(jt + 1) * C],
                                        ident[:],
                                        start=False, stop=(jt == jts[-1]),
                                        tile_position=(0, C))
                        x0 = n0 // H
                        nx = (n1 - n0) // H
                        for m in range(2):
                            nc.vector.tensor_tensor(
                                out=fa3[m * C:(m + 1) * C,
                                        x0:x0 + nx, 1:H + 1],
                                in0=psm[m][m * C:(m + 1) * C, 0:n1 - n0],
                                in1=s_bc[m * C:(m + 1) * C, n0:n1],
                                op=OP.mult)

                    def conv2_q(q):
                        # one 8-row quarter of the output (needs conv1
                        # chunks q and q+1 for its fused-row window)
                        half = q % 2
                        co = slice(half * C, (half + 1) * C)
                        psm = cvp.tile([128, 512], F32,
                                       tag=("c2a" if half == 0 else "c2b"),
                                       name="c2q")
                        ox = 1 + q * 8
                        for t in range(9):
                            dx, dy = t // 3, t % 3
                            ra = fa3[:, ox + dx - 1:ox + dx + 7,
                                     dy:dy + H]
                            rb = fb3[:, ox + dx - 1:ox + dx + 7,
                                     dy:dy + H]
                            nc.tensor.matmul(
                                psm[co, :],
                                w2a_t[:, t * C:(t + 1) * C], ra,
                                start=(t == 0), stop=False,
                                tile_position=(0, half * C))
                            nc.tensor.matmul(
                                psm[co, :],
                                w2b_t[:, t * C:(t + 1) * C], rb,
                                start=False, stop=(t == 8),
                                tile_position=(0, half * C))
                        ob = slice((q // 2) * 512, (q // 2) * 512 + 512)
                        nc.scalar.activation(
                            out_sb[co, ob],
                            psm[co, :], AF.Relu,
                            bias=bnb_t[co, 0:1], scale=bna_t[co, 0:1])
                        y3 = yout[:].rearrange("c (x y) -> c x y", y=H)
                        nc.sync.dma_start(
                            out=y3[:, q * 8:q * 8 + 8, :],
                            in_=out_sb[co, ob]
                            .rearrange("c (x y) -> c x y", y=H))

                    # Software-pipelined emission: group g's s_group/conv1
                    # are emitted after group g+1's spine, so their data deps
                    # (gathers, S transposes) are a full group stale and
                    # never head-block the in-order PE/Pool queues.  Gathers
                    # are deferred 2 match-tiles for the same reason.
                    def flush_gathers(keep=0):
                        while len(pend_gather) > keep:
                            pend_gather.pop(0)()

                    def flush_counts(keep=0):
                        while len(pend_count) > keep:
                            pend_count.pop(0)()

                    groups = [list(range(4 * g, min(4 * g + 4, JT)))
                              for g in range(5)]
                    for g in range(5):
                        for jt in groups[g]:
                            for m in range(2):
                                # two units of G' work interleaved per
                                # match-tile keeps startup non-serial
                                for _ in range(2):
                                    if g_work:
                                        g_work.pop(0)()
                                i2 = 2 * jt + m
                                use_act = ((i2 + 1) * 12 // 34) > \
                                    (i2 * 12 // 34)
                                spine_mjt(m, jt, use_act)
                                flush_counts(keep=1)
                                # gt_dram is complete after group 0; hold
                                # gathers until then so Pool never stalls
                                flush_gathers(keep=2 if g >= 1 else 99)
                        if g >= 1:
                            s_group(g - 1, groups[g - 1])
                            conv1_chunk(g - 1)
                        if g == 4:
                            conv2_q(0)
                            conv2_q(1)
                    flush_counts(keep=0)
                    flush_gathers(keep=0)
                    s_group(4, groups[4])
                    conv1_chunk(4)
                    conv2_q(2)
                    conv2_q(3)

    nc.finalize()
    return nc


_NC_CACHE = None


def _get_nc():
    global _NC_CACHE
    if _NC_CACHE is None:
        _NC_CACHE = _build_nc()
    return _NC_CACHE


def _bf16_split(x):
    hi = x.astype(ml_dtypes.bfloat16)
    lo = (x - hi.astype(np.float32)).astype(ml_dtypes.bfloat16)
    return hi, lo


def _host_prep(inputs):
    V = np.ascontiguousarray(inputs["V_rgb"], dtype=np.float32)
    K = np.ascontiguousarray(inputs["K_dep"], dtype=np.float32)
    Q = np.ascontiguousarray(inputs["Q_flo"], dtype=np.float32)
    w1 = np.asarray(inputs["conv1_w"], dtype=np.float32)[:, :, 0, 0]
    b1 = np.asarray(inputs["conv1_b"], dtype=np.float32)
    w2 = np.asarray(inputs["conv2_w"], dtype=np.float32)
    b2 = np.asarray(inputs["conv2_b"], dtype=np.float32)
    g = np.asarray(inputs["bn_gamma"], dtype=np.float32)
    be = np.asarray(inputs["bn_beta"], dtype=np.float32)
    mu = np.asarray(inputs["bn_mean"], dtype=np.float32)
    var = np.asarray(inputs["bn_var"], dtype=np.float32)

    w1vt = np.ascontiguousarray(w1[:, :C].T).astype(ml_dtypes.bfloat16)
    w1tt1 = np.ascontiguousarray(w1[:, C:].T)
    w1tt = np.concatenate([w1tt1, w1tt1], axis=0).astype(ml_dtypes.bfloat16)
    w2a = np.zeros((128, 9 * C), np.float32)
    w2b = np.zeros((C, 9 * C), np.float32)
    for t in range(9):
        dx, dy = t // 3, t % 3
        lhsT = w2[:, :, dx, dy].T                     # [192, 64]
        w2a[:, t * C:(t + 1) * C] = lhsT[0:128]
        w2b[:, t * C:(t + 1) * C] = lhsT[128:192]
    w2a = w2a.astype(ml_dtypes.bfloat16)
    w2b = w2b.astype(ml_dtypes.bfloat16)
    bna = g / np.sqrt(var + BN_EPS)
    bnb = be + (b2 - mu) * bna
    bna2 = np.ascontiguousarray(np.concatenate([bna, bna])[:, None])
    bnb2 = np.ascontiguousarray(np.concatenate([bnb, bnb])[:, None])

    in_maps = []
    for core in range(N_CORES):
        b, half = core // 2, core % 2
        x0 = half * (W // 2)
        vw = np.zeros((C, WROWS, H), np.float32)
        lo = x0 - 1
        hi = x0 + W // 2 + 1
        slo, shi = max(lo, 0), min(hi, W)
        vw[:, slo - lo:slo - lo + (shi - slo), :] = V[b, :, slo:shi, :]
        vw = vw.reshape(C, JW)
        aq = Q[b].reshape(C, HW)
        ak = K[b].reshape(C, HW)
        aq1, aq2 = _bf16_split(aq)
        ak1, ak2 = _bf16_split(ak)
        v1, v2 = _bf16_split(vw)
        ahi = np.concatenate([aq1, ak1], axis=0)      # [128, HW]
        axq = np.concatenate([aq2, aq1], axis=0)
        axk = np.concatenate([ak2, ak1], axis=0)
        vhi = np.concatenate([v1, v1], axis=0)        # [128, JW]
        vx = np.concatenate([v1, v2], axis=0)
        in_maps.append({
            "ahi": np.ascontiguousarray(ahi),
            "axq": np.ascontiguousarray(axq),
            "axk": np.ascontiguousarray(axk),
            "vhi": np.ascontiguousarray(vhi),
            "vx": np.ascontiguousarray(vx),
            "vwin": np.ascontiguousarray(vw.astype(ml_dtypes.bfloat16)),
            "w1vt": w1vt,
            "w1tt": w1tt,
            "b1d": np.ascontiguousarray(b1[:, None]),
            "w2ad": w2a,
            "w2bd": w2b,
            "bnad": bna2,
            "bnbd": bnb2,
        })
    return in_maps


def kernel(**inputs):
    nc = _get_nc()
    in_maps = _host_prep(inputs)
    res = bass_utils.run_bass_kernel_spmd(
        nc, in_maps, core_ids=list(range(N_CORES)))
    y = np.zeros((B, C, W, H), np.float32)
    for core in range(N_CORES):
        b, half = core // 2, core % 2
        x0 = half * (W // 2)
        y[b, :, x0:x0 + W // 2, :] = \
            res.results[core]["y"].reshape(C, OUT_ROWS, H)
    return y


# revision 74
# speedup vs baseline: 1.0008x; 1.0008x over previous
"""Trainium2 Bass kernel for nn_Attention_block (retrieval_knn).

Reference (per sample b, match A in {Q_flo, K_dep}, V = V_rgb):
  T[i,j] = <A[:,i], V[:,j]>          [4096, 4096] score matrix
  S[j] = max_i T ; idx[j] = argmax_i T
  C = conv1x1([V; A[:, idx]]) * S    (conv1: 128->64)
  fused = [C_v, C_k, V]              (192 ch)
  y = relu(BN(conv3x3(fused)))       (conv2: 192->64, pad 1)

Sharding: 8 cores = 4 samples x 2 W-halves (pure data parallel; each core
takes a 1-row halo each side of its half for the 3x3 conv and computes its
2176 j-columns against the full 4096-long i axis).

Device-side structure per core (252.5us cost model, 1.41x over the
357.1us fp32 baseline; DVE/ACT-bound):
  - G-trick: gather commutes with conv1's TA half:
      conv1([V;TA]) + b1 = W1v@V + (W1t@A + b1)[:, idx]
    G' = W1t@A + b1 is computed once per match, transposed into DRAM
    [4096, 64], and argmax rows are fetched by indirect-DMA gather.
  - Scores must be fp32-exact: smallest top-2 score gap here is ~1.5e-4 and
    the reference argmax is f32.  Plain fp32 matmuls cost 4 PE cycles/row
    and fp32r is TF32 (10-bit - flips argmaxes), so the spine uses an exact
    bf16 split computed on the host: A = A1 + A2, V = V1 + V2 (hi/lo bf16
    pairs); T = V1'A1 + (V1'A2 + V2'A1), where the two cross terms fold
    into ONE K=128 matmul by stacking [V1;V2] against [A2;A1] (PE cost is
    per output row, independent of K) - 2 bf16 matmuls/chunk instead of 1
    fp32 at 4 cyc/row.  |T~ - T| ~ 3e-5, 5x below the min gap.
  - Argmax spine, m-major per 128-j tile: chunk matmuls accumulate into
    2-bank PSUM pair tiles; ACT evacuates [128,1024] pairs into an SBUF row
    T_sb [128,4096].  The max+index extraction alternates two formulations
    to balance DVE vs ACT (gpsimd cannot run TensorScalarPtr, so Pool only
    gets gathers/iota/copies):
      * DVE-style (22 tiles): 2x-mode max pass (tensor_scalar accum -> S)
        + 1x is_equal*iota pass (scalar_tensor_tensor accum -> idx).
      * ACT-style (12 tiles): DVE prefix-max scan (tensor_tensor_scan
        max/bypass, in place), S = P[:,-1] (Pool copies), then
        idx = sum(sign(S - P)) as ONE ACT Sign activation (scale=-1,
        per-partition bias = P[:,-1] = S read straight from the scan
        output) with the sum accumulator.  Exact, first-occurrence tie
        semantics.  (Emitting this ACT work deferred breaks real-HW
        correctness - keep it inline.)
  - Software-pipelined emission: group g's s_group/conv1 are emitted after
    group g+1's spine so gathers/S-broadcasts are a full group stale and
    never head-block the in-order PE/Pool queues; gathers are deferred 2
    match-tiles; G' work is interleaved into spine group 0; conv2 runs as
    8-row quarters, two at g==4 and two after the last group.
  - conv1 / conv2 / G' matmuls run in bf16 (value paths; ~2.9e-3 rel err,
    well under the 2e-2 gate).  The G'->transpose->gather->conv1-TA chain
    stays fp32 (transposes must dtype-match their data, and a 16-bit
    transpose output cannot accumulate into an fp32 PSUM group).
  - PSUM (8 banks): spine 2x[128,1024] pairs (4) + conv1 cva/cvb (2) +
    conv2/transpose-staging c2a/c2b (2).  Sharing one bank between two
    64-partition accumulation groups passes the simulator but breaks on
    hardware (start=True zeroes the whole region) - keep halves in
    separate banks.
"""

import numpy as np
import ml_dtypes

import concourse.bass as bass
import concourse.bacc as bacc
import concourse.mybir as mybir
from concourse.tile import TileContext
from concourse import bass_utils
from concourse.masks import make_identity

F32 = mybir.dt.float32
BF16 = mybir.dt.bfloat16
I32 = mybir.dt.int32
AF = mybir.ActivationFunctionType
OP = mybir.AluOpType

B, C, W, H = 4, 64, 64, 64
HW = W * H                     # 4096
BN_EPS = 1e-5
N_CORES = 8
WROWS = W // 2 + 2             # 34 window rows (half + 1-row halo each side)
JW = WROWS * H                 # 2176 j-columns per core
JT = JW // 128                 # 17 j-tiles
NCH = HW // 512                # 8 i-chunks
OUT_ROWS = W // 2              # 32 interior rows per core
YPAD = H + 2                   # 66 padded y positions in fused layout

NEG = -3.0e38


def _build_nc():
    nc = bacc.Bacc("TRN2", target_bir_lowering=False)

    # Exact bf16 split pairs (X = X1 + X2 with X2 itself bf16-exact to
    # ~2^-17 rel).  T = V1'A1 + (V1'A2 + V2'A1): the two cross terms are
    # computed as ONE K=128 matmul by stacking [V1;V2] against [A2;A1] -
    # PE cost is per output row, independent of K.
    #   ahi: rows 0:64 = A1_q, rows 64:128 = A1_k
    #   axq/axk: rows 0:64 = A2_m, rows 64:128 = A1_m  (match m)
    #   vhi: V1 duplicated on both row halves; vx: rows 0:64 V1, 64:128 V2
    ahi = nc.dram_tensor("ahi", [128, HW], BF16, kind="ExternalInput")
    axq = nc.dram_tensor("axq", [128, HW], BF16, kind="ExternalInput")
    axk = nc.dram_tensor("axk", [128, HW], BF16, kind="ExternalInput")
    vhi = nc.dram_tensor("vhi", [128, JW], BF16, kind="ExternalInput")
    vx = nc.dram_tensor("vx", [128, JW], BF16, kind="ExternalInput")
    vwin = nc.dram_tensor("vwin", [C, JW], BF16, kind="ExternalInput")
    w1vt = nc.dram_tensor("w1vt", [C, C], BF16, kind="ExternalInput")
    w1tt = nc.dram_tensor("w1tt", [128, C], BF16, kind="ExternalInput")
    b1d = nc.dram_tensor("b1d", [C, 1], F32, kind="ExternalInput")
    w2ad = nc.dram_tensor("w2ad", [128, 9 * C], BF16, kind="ExternalInput")
    w2bd = nc.dram_tensor("w2bd", [C, 9 * C], BF16, kind="ExternalInput")
    bnad = nc.dram_tensor("bnad", [128, 1], F32, kind="ExternalInput")
    bnbd = nc.dram_tensor("bnbd", [128, 1], F32, kind="ExternalInput")
    yout = nc.dram_tensor("y", [C, OUT_ROWS * H], F32, kind="ExternalOutput")

    if True:
      with TileContext(nc) as tc:
        with tc.tile_pool(name="persist", bufs=1) as pp:
            ahi_t = pp.tile([128, HW], BF16)
            axq_t = pp.tile([128, HW], BF16)
            axk_t = pp.tile([128, HW], BF16)
            vhi_t = pp.tile([128, JW], BF16)
            vx_t = pp.tile([128, JW], BF16)
            w1vt_t = pp.tile([C, C], BF16)
            w1tt_t = pp.tile([128, C], BF16)
            b1_t = pp.tile([C, 1], F32)
            w2a_t = pp.tile([128, 9 * C], BF16)
            w2b_t = pp.tile([C, 9 * C], BF16)
            bna_t = pp.tile([128, 1], F32)
            bnb_t = pp.tile([128, 1], F32)
            iota_t = pp.tile([128, HW], F32)
            ident = pp.tile([128, 128], F32)
            gtile = [pp.tile([128, JT * C], F32, tag="gtq", name="gtq_t"),
                     pp.tile([128, JT * C], F32, tag="gtk", name="gtk_t")]
            s_all = [pp.tile([128, JT], F32, tag="sq", name="sq_t"),
                     pp.tile([128, JT], F32, tag="sk", name="sk_t")]
            idx_all = [pp.tile([128, JT], I32, tag="idxq", name="idxq_t"),
                       pp.tile([128, JT], I32, tag="idxk", name="idxk_t")]
            s_bc = pp.tile([128, JW], F32)     # rows 0:64 Sq, 64:128 Sk
            fused_a = pp.tile([128, WROWS * YPAD], BF16)  # C_v / C_k
            fused_b = pp.tile([C, WROWS * YPAD], BF16)    # V, y-padded
            out_sb = pp.tile([128, OUT_ROWS * H // 2], F32)

            # load order: the first spine tile needs only ahi/vhi/axq
            nc.sync.dma_start(out=ahi_t[:], in_=ahi[:])
            nc.sync.dma_start(out=vhi_t[:], in_=vhi[:])
            nc.sync.dma_start(out=axq_t[:], in_=axq[:])
            nc.sync.dma_start(out=vx_t[:], in_=vx[:])
            nc.sync.dma_start(out=axk_t[:], in_=axk[:])
            nc.sync.dma_start(out=w1vt_t[:], in_=w1vt[:])
            nc.sync.dma_start(out=w1tt_t[:], in_=w1tt[:])
            nc.sync.dma_start(out=b1_t[:], in_=b1d[:])
            nc.sync.dma_start(out=w2a_t[:], in_=w2ad[:])
            nc.sync.dma_start(out=w2b_t[:], in_=w2bd[:])
            nc.sync.dma_start(out=bna_t[:], in_=bnad[:])
            nc.sync.dma_start(out=bnb_t[:], in_=bnbd[:])
            nc.gpsimd.iota(iota_t[:], pattern=[[1, HW]], base=0,
                           channel_multiplier=0,
                           allow_small_or_imprecise_dtypes=True)
            make_identity(nc, ident[:])

            fb3 = fused_b[:].rearrange("c (x y) -> c x y", y=YPAD)
            nc.gpsimd.memset(fused_b[:], 0.0)
            nc.sync.dma_start(
                out=fb3[:, :, 1:H + 1],
                in_=vwin[:].rearrange("c (x y) -> c x y", y=H))
            nc.gpsimd.memset(gtile[0][:], 0.0)
            nc.gpsimd.memset(gtile[1][:], 0.0)

            with tc.tile_pool(name="gdram", bufs=1, space="DRAM") as gdr:
                gt_dram = [gdr.tile([HW, C], F32, tag="gtdq", name="gtdq_t"),
                           gdr.tile([HW, C], F32, tag="gtdk", name="gtdk_t")]

                # ---- Phases 3-6 interleaved ----
                fa3 = fused_a[:].rearrange("c (x y) -> c x y", y=YPAD)
                nc.gpsimd.memset(fa3[:, :, 0:1], 0.0)
                nc.gpsimd.memset(fa3[:, :, YPAD - 1:YPAD], 0.0)
                with tc.tile_pool(name="sp_ps", bufs=1, space="PSUM") as sps, \
                     tc.tile_pool(name="sp_sb", bufs=2) as ssb, \
                     tc.tile_pool(name="sp_sm", bufs=4) as ssm, \
                     tc.tile_pool(name="cv_ps", bufs=1, space="PSUM") as cvp, \
                     tc.tile_pool(name="s4_sb", bufs=2) as s4, \
                     tc.tile_pool(name="s4_dram", bufs=1, space="DRAM") as d4:

                    # ---- Phase 1+2: G' = W1t @ A1 + b1; transpose to DRAM.
                    # Emitted as closures interleaved into spine group 0 so
                    # the startup isn't serial (spine doesn't depend on G').
                    g_sbs = [pp.tile([C, HW], F32, tag="gsbq", name="gsbq"),
                             pp.tile([C, HW], F32, tag="gsbk", name="gsbk")]

                    def g_chunk(c8):
                        pms = [cvp.tile([C, 512], F32, tag="cva", name="gmq"),
                               cvp.tile([C, 512], F32, tag="cvb", name="gmk")]
                        for m in range(2):
                            ro = m * C
                            nc.tensor.matmul(
                                pms[m][:], w1tt_t[ro:ro + C, :],
                                ahi_t[ro:ro + C, c8 * 512:(c8 + 1) * 512],
                                start=True, stop=True,
                                tile_position=(ro, 0))
                        for m in range(2):
                            nc.scalar.activation(
                                g_sbs[m][:, c8 * 512:(c8 + 1) * 512],
                                pms[m][:],
                                AF.Identity, bias=b1_t[:, 0:1], scale=1.0)

                    def g_transpose(m, grp):
                        g_sb = g_sbs[m]
                        pst = cvp.tile([128, 512], F32, tag="c2a",
                                       name="gtr")
                        stg = pp.tile([128, 512], F32, tag="stg")
                        for t in range(8):
                            blk = grp * 8 + t
                            nc.tensor.matmul(
                                pst[:, t * C:(t + 1) * C],
                                g_sb[:, blk * 128:(blk + 1) * 128],
                                ident[0:C, 0:C], is_transpose=True,
                                start=True, stop=True)
                        nc.scalar.copy(stg[:], pst[:])
                        nc.sync.dma_start(
                            out=gt_dram[m][:]
                            .rearrange("(g p) c -> p g c", p=128)
                            [:, grp * 8:(grp + 1) * 8, :],
                            in_=stg[:].rearrange("p (g c) -> p g c", c=C))

                    g_work = [lambda c8=c8: g_chunk(c8) for c8 in range(NCH)]
                    g_work += [lambda m=m, grp=grp: g_transpose(m, grp)
                               for m in range(2) for grp in range(4)]

                    pend_gather = []
                    pend_count = []
                    neg1_t = pp.tile([128, 1], F32, tag="neg1",
                                     name="neg1")
                    nc.gpsimd.memset(neg1_t[:], -1.0)

                    def spine_mjt(m, jt, use_act):
                        # One match x one 128-j tile: 8 i-chunk matmul
                        # triples into 2-bank PSUM pair tiles, ACT evacuates
                        # [128,1024] pairs into T_sb, DVE does the 2x max
                        # pass, and the 1x is_equal*iota index pass runs on
                        # DVE or Pool (gpsimd) per the balance schedule.
                        tsb = ssb.tile([128, HW], F32, tag=f"tsb{m}",
                                       name=f"tsb{m}")
                        ro = m * C
                        js = slice(jt * 128, (jt + 1) * 128)
                        ax_t = axq_t if m == 0 else axk_t
                        for pr in range(4):
                            ps = sps.tile([128, 1024], F32,
                                          tag=f"sp{pr % 2}", name="sp")
                            for h in range(2):
                                ch = pr * 2 + h
                                cs = slice(ch * 512, (ch + 1) * 512)
                                o = slice(h * 512, (h + 1) * 512)
                                nc.tensor.matmul(
                                    ps[:, o], vhi_t[ro:ro + C, js],
                                    ahi_t[ro:ro + C, cs],
                                    start=True, stop=False,
                                    tile_position=(ro, 0))
                                nc.tensor.matmul(
                                    ps[:, o], vx_t[:, js],
                                    ax_t[:, cs],
                                    start=False, stop=True)
                            nc.scalar.copy(
                                tsb[:, pr * 1024:(pr + 1) * 1024], ps[:])
                        S = s_all[m][:, jt:jt + 1]
                        if use_act:
                            # Offload the index pass to ScalarE: P =
                            # prefix-max(T) (DVE scan, in place), then
                            # idx = -sum(sign(P - S)) via one ACT Sign
                            # activation with per-partition bias -S and the
                            # sum accumulator.  Exact, first-occurrence.
                            # The ACT part is emitted one match-tile later
                            # (via pend_count) so ACT's in-order queue never
                            # waits on the scan.
                            nc.vector.tensor_tensor_scan(
                                out=tsb[:], data0=tsb[:], data1=tsb[:],
                                initial=NEG, op0=OP.max, op1=OP.bypass)
                            nia = ssm.tile([128, 1], F32, tag="nia",
                                           name="nia")
                            nc.gpsimd.tensor_copy(
                                S, tsb[:, HW - 1:HW])
                            # idx = sum(sign(S - P)): scale=-1 with bias=S
                            # read straight from the scan's last column, so
                            # the accumulated count is directly positive.
                            nc.scalar.activation(
                                tsb[:], tsb[:], AF.Sign,
                                bias=tsb[:, HW - 1:HW], scale=-1.0,
                                accum_out=nia[:])
                            nc.scalar.activation(
                                idx_all[m][:, jt:jt + 1], nia[:],
                                AF.Copy, bias=0.0, scale=1.0)
                        else:
                            nc.vector.tensor_scalar(
                                out=tsb[:], in0=tsb[:],
                                scalar1=NEG, scalar2=NEG,
                                op0=OP.max, op1=OP.max, accum_out=S)
                            ist = ssm.tile([128, 1], F32, tag="ist",
                                           name="ist")
                            nc.vector.scalar_tensor_tensor(
                                out=tsb[:], in0=tsb[:], scalar=S,
                                in1=iota_t[:], op0=OP.is_equal,
                                op1=OP.mult, accum_out=ist[:])
                            nc.vector.tensor_copy(
                                idx_all[m][:, jt:jt + 1], ist[:])

                        def fire(m=m, jt=jt):
                            nc.gpsimd.indirect_dma_start(
                                out=gtile[m][:, jt * C:(jt + 1) * C],
                                out_offset=None,
                                in_=gt_dram[m][:],
                                in_offset=bass.IndirectOffsetOnAxis(
                                    ap=idx_all[m][:, jt:jt + 1], axis=0),
                                bounds_check=HW - 1, oob_is_err=False)
                        pend_gather.append(fire)

                    s_dram = [d4.tile([JW], F32, tag="sdq", name="sdq"),
                              d4.tile([JW], F32, tag="sdk", name="sdk")]

                    def s_group(g, jts):
                        n0 = jts[0] * 128
                        n1 = (jts[-1] + 1) * 128
                        nt = len(jts)
                        for m in range(2):
                            pst = cvp.tile([nt, 128], F32, tag="c2a",
                                           name="pst")
                            nc.tensor.matmul(
                                pst[:], s_all[m][:, jts[0]:jts[-1] + 1],
                                ident[:], is_transpose=True,
                                start=True, stop=True)
                            stg = s4.tile([JT, 128], F32, tag="stg4",
                                          name="stg4")
                            nc.scalar.copy(stg[0:nt, :], pst[:])
                            nc.sync.dma_start(
                                out=s_dram[m][n0:n1]
                                .rearrange("(t p) -> t p", p=128),
                                in_=stg[0:nt, :])
                            nc.sync.dma_start(
                                out=s_bc[m * C:(m + 1) * C, n0:n1],
                                in_=s_dram[m][None, n0:n1]
                                .to_broadcast((C, n1 - n0)))

                    def conv1_chunk(cn):
                        jts = list(range(4 * cn, min(4 * cn + 4, JT)))
                        n0 = cn * 512
                        n1 = min(n0 + 512, JW)
                        psm = [cvp.tile([128, 512], F32, tag="cva",
                                        name="cva"),
                               cvp.tile([128, 512], F32, tag="cvb",
                                        name="cvb")]
                        for m in range(2):
                            nc.tensor.matmul(
                                psm[m][m * C:(m + 1) * C, 0:n1 - n0],
                                w1vt_t[:], vhi_t[0:C, n0:n1],
                                start=True, stop=False,
                                tile_position=(0, m * C))
                        for m in range(2):
                            for i, jt in enumerate(jts):
                                if m == 0:
                                    nc.tensor.matmul(
                                        psm[m][0:C, i * 128:(i + 1) * 128],
                                        gtile[m][:, jt * C:(jt + 1) * C],
                                        ident[:], is_transpose=True,
                                        start=False, stop=(jt == jts[-1]))
                                else:
                                    nc.tensor.matmul(
                                        psm[m][C:128,
                                               i * 128:(i + 1) * 128],
                                        gtile[m][:, jt * C:(jt + 1) * C],
                                        ident[:],
                                        start=False, stop=(jt == jts[-1]),
                                        tile_position=(0, C))
                        x0 = n0 // H
                        nx = (n1 - n0) // H
                        for m in range(2):
                            nc.vector.tensor_tensor(
                                out=fa3[m * C:(m + 1) * C,
                                        x0:x0 + nx, 1:H + 1],
                                in0=psm[m][m * C:(m + 1) * C, 0:n1 - n0],
                                in1=s_bc[m * C:(m + 1) * C, n0:n1],
                                op=OP.mult)

                    def conv2_q(q):
                        # one 8-row quarter of the output (needs conv1
                        # chunks q and q+1 for its fused-row window)
                        half = q % 2
                        co = slice(half * C, (half + 1) * C)
                        psm = cvp.tile([128, 512], F32,
                                       tag=("c2a" if half == 0 else "c2b"),
                                       name="c2q")
                        ox = 1 + q * 8
                        for t in range(9):
                            dx, dy = t // 3, t % 3
                            ra = fa3[:, ox + dx - 1:ox + dx + 7,
                                     dy:dy + H]
                            rb = fb3[:, ox + dx - 1:ox + dx + 7,
                                     dy:dy + H]
                            nc.tensor.matmul(
                                psm[co, :],
                                w2a_t[:, t * C:(t + 1) * C], ra,
                                start=(t == 0), stop=False,
                                tile_position=(0, half * C))
                            nc.tensor.matmul(
                                psm[co, :],
                                w2b_t[:, t * C:(t + 1) * C], rb,
                                start=False, stop=(t == 8),
                                tile_position=(0, half * C))
                        ob = slice((q // 2) * 512, (q // 2) * 512 + 512)
                        nc.scalar.activation(
                            out_sb[co, ob],
                            psm[co, :], AF.Relu,
                            bias=bnb_t[co, 0:1], scale=bna_t[co, 0:1])
                        y3 = yout[:].rearrange("c (x y) -> c x y", y=H)
                        nc.sync.dma_start(
                            out=y3[:, q * 8:q * 8 + 8, :],
                            in_=out_sb[co, ob]
                            .rearrange("c (x y) -> c x y", y=H))

                    # Software-pipelined emission: group g's s_group/conv1
                    # are emitted after group g+1's spine, so their data deps
                    # (gathers, S transposes) are a full group stale and
                    # never head-block the in-order PE/Pool queues.  Gathers
                    # are deferred 2 match-tiles for the same reason.
                    def flush_gathers(keep=0):
                        while len(pend_gather) > keep:
                            pend_gather.pop(0)()

                    def flush_counts(keep=0):
                        while len(pend_count) > keep:
                            pend_count.pop(0)()

                    groups = [list(range(4 * g, min(4 * g + 4, JT)))
                              for g in range(5)]
                    for g in range(5):
                        for jt in groups[g]:
                            for m in range(2):
                                # two units of G' work interleaved per
                                # match-tile keeps startup non-serial
                                for _ in range(2):
                                    if g_work:
                                        g_work.pop(0)()
                                i2 = 2 * jt + m
                                use_act = ((i2 + 1) * 12 // 34) > \
                                    (i2 * 12 // 34)
                                spine_mjt(m, jt, use_act)
                                flush_counts(keep=1)
                                # gt_dram is complete after group 0; hold
                                # gathers until then so Pool never stalls
                                flush_gathers(keep=2 if g >= 1 else 99)
                        if g >= 1:
                            s_group(g - 1, groups[g - 1])
                            conv1_chunk(g - 1)
                        if g == 4:
                            conv2_q(0)
                            conv2_q(1)
                    flush_counts(keep=0)
                    flush_gathers(keep=0)
                    s_group(4, groups[4])
                    conv1_chunk(4)
                    conv2_q(2)
                    conv2_q(3)

    nc.finalize()
    return nc


_NC_CACHE = None


def _get_nc():
    global _NC_CACHE
    if _NC_CACHE is None:
        _NC_CACHE = _build_nc()
    return _NC_CACHE


def _bf16_split(x):
    hi = x.astype(ml_dtypes.bfloat16)
    lo = (x - hi.astype(np.float32)).astype(ml_dtypes.bfloat16)
    return hi, lo


def _host_prep(inputs):
    V = np.ascontiguousarray(inputs["V_rgb"], dtype=np.float32)
    K = np.ascontiguousarray(inputs["K_dep"], dtype=np.float32)
    Q = np.ascontiguousarray(inputs["Q_flo"], dtype=np.float32)
    w1 = np.asarray(inputs["conv1_w"], dtype=np.float32)[:, :, 0, 0]
    b1 = np.asarray(inputs["conv1_b"], dtype=np.float32)
    w2 = np.asarray(inputs["conv2_w"], dtype=np.float32)
    b2 = np.asarray(inputs["conv2_b"], dtype=np.float32)
    g = np.asarray(inputs["bn_gamma"], dtype=np.float32)
    be = np.asarray(inputs["bn_beta"], dtype=np.float32)
    mu = np.asarray(inputs["bn_mean"], dtype=np.float32)
    var = np.asarray(inputs["bn_var"], dtype=np.float32)

    w1vt = np.ascontiguousarray(w1[:, :C].T).astype(ml_dtypes.bfloat16)
    w1tt1 = np.ascontiguousarray(w1[:, C:].T)
    w1tt = np.concatenate([w1tt1, w1tt1], axis=0).astype(ml_dtypes.bfloat16)
    w2a = np.zeros((128, 9 * C), np.float32)
    w2b = np.zeros((C, 9 * C), np.float32)
    for t in range(9):
        dx, dy = t // 3, t % 3
        lhsT = w2[:, :, dx, dy].T                     # [192, 64]
        w2a[:, t * C:(t + 1) * C] = lhsT[0:128]
        w2b[:, t * C:(t + 1) * C] = lhsT[128:192]
    w2a = w2a.astype(ml_dtypes.bfloat16)
    w2b = w2b.astype(ml_dtypes.bfloat16)
    bna = g / np.sqrt(var + BN_EPS)
    bnb = be + (b2 - mu) * bna
    bna2 = np.ascontiguousarray(np.concatenate([bna, bna])[:, None])
    bnb2 = np.ascontiguousarray(np.concatenate([bnb, bnb])[:, None])

    in_maps = []
    for core in range(N_CORES):
        b, half = core // 2, core % 2
        x0 = half * (W // 2)
        vw = np.zeros((C, WROWS, H), np.float32)
        lo = x0 - 1
        hi = x0 + W // 2 + 1
        slo, shi = max(lo, 0), min(hi, W)
        vw[:, slo - lo:slo - lo + (shi - slo), :] = V[b, :, slo:shi, :]
        vw = vw.reshape(C, JW)
        aq = Q[b].reshape(C, HW)
        ak = K[b].reshape(C, HW)
        aq1, aq2 = _bf16_split(aq)
        ak1, ak2 = _bf16_split(ak)
        v1, v2 = _bf16_split(vw)
        ahi = np.concatenate([aq1, ak1], axis=0)      # [128, HW]
        axq = np.concatenate([aq2, aq1], axis=0)
        axk = np.concatenate([ak2, ak1], axis=0)
        vhi = np.concatenate([v1, v1], axis=0)        # [128, JW]
        vx = np.concatenate([v1, v2], axis=0)
        in_maps.append({
            "ahi": np.ascontiguousarray(ahi),
            "axq": np.ascontiguousarray(axq),
            "axk": np.ascontiguousarray(axk),
            "vhi": np.ascontiguousarray(vhi),
            "vx": np.ascontiguousarray(vx),
            "vwin": np.ascontiguousarray(vw.astype(ml_dtypes.bfloat16)),
            "w1vt": w1vt,
            "w1tt": w1tt,
            "b1d": np.ascontiguousarray(b1[:, None]),
            "w2ad": w2a,
            "w2bd": w2b,
            "bnad": bna2,
            "bnbd": bnb2,
        })
    return in_maps


def kernel(**inputs):
    nc = _get_nc()
    in_maps = _host_prep(inputs)
    res = bass_utils.run_bass_kernel_spmd(
        nc, in_maps, core_ids=list(range(N_CORES)))
    y = np.zeros((B, C, W, H), np.float32)
    for core in range(N_CORES):
        b, half = core // 2, core % 2
        x0 = half * (W // 2)
        y[b, :, x0:x0 + W // 2, :] = \
            res.results[core]["y"].reshape(C, OUT_ROWS, H)
    return y


# revision 83
# speedup vs baseline: 1.0364x; 1.0356x over previous
"""Trainium2 Bass kernel for nn_Attention_block (retrieval_knn).

Reference (per sample b, match A in {Q_flo, K_dep}, V = V_rgb):
  T[i,j] = <A[:,i], V[:,j]>          [4096, 4096] score matrix
  S[j] = max_i T ; idx[j] = argmax_i T
  C = conv1x1([V; A[:, idx]]) * S    (conv1: 128->64)
  fused = [C_v, C_k, V]              (192 ch)
  y = relu(BN(conv3x3(fused)))       (conv2: 192->64, pad 1)

Sharding: 8 cores = 4 samples x 2 W-halves (pure data parallel; each core
takes a 1-row halo each side of its half for the 3x3 conv and computes its
2176 j-columns against the full 4096-long i axis).

Device-side structure per core (243.8us cost model, 1.46x over the
357.1us fp32 baseline; DVE/ACT-bound):
  - G-trick: gather commutes with conv1's TA half:
      conv1([V;TA]) + b1 = W1v@V + (W1t@A + b1)[:, idx]
    G' = W1t@A + b1 is computed once per match, transposed into DRAM
    [4096, 64], and argmax rows are fetched by indirect-DMA gather.
  - Scores must be fp32-exact: smallest top-2 score gap here is ~1.5e-4 and
    the reference argmax is f32.  Plain fp32 matmuls cost 4 PE cycles/row
    and fp32r is TF32 (10-bit - flips argmaxes), so the spine uses an exact
    bf16 split computed on the host: A = A1 + A2, V = V1 + V2 (hi/lo bf16
    pairs); T = V1'A1 + (V1'A2 + V2'A1), where the two cross terms fold
    into ONE K=128 matmul by stacking [V1;V2] against [A2;A1] (PE cost is
    per output row, independent of K) - 2 bf16 matmuls/chunk instead of 1
    fp32 at 4 cyc/row.  |T~ - T| ~ 3e-5, 5x below the min gap.
  - Argmax spine, m-major per 128-j tile: chunk matmuls accumulate into
    2-bank PSUM pair tiles; ACT evacuates [128,1024] pairs into an SBUF row
    T_sb [128,4096].  The max+index extraction alternates two formulations
    to balance DVE vs ACT (gpsimd cannot run TensorScalarPtr, so Pool only
    gets gathers/iota/copies):
      * DVE-style (22 tiles): 2x-mode max pass (tensor_scalar accum -> S)
        + 1x is_equal*iota pass (scalar_tensor_tensor accum -> idx).
      * ACT-style (12 tiles): DVE prefix-max scan (tensor_tensor_scan
        max/bypass, in place), S = P[:,-1] (Pool copies), then
        idx = sum(sign(S - P)) as ONE ACT Sign activation (scale=-1,
        per-partition bias = P[:,-1] = S read straight from the scan
        output) with the sum accumulator.  Exact, first-occurrence tie
        semantics.  (Emitting this ACT work deferred breaks real-HW
        correctness - keep it inline.)
  - Software-pipelined emission: group g's s_group/conv1 are emitted after
    group g+1's spine so gathers/S-broadcasts are a full group stale and
    never head-block the in-order PE/Pool queues; gathers are deferred 2
    match-tiles; G' work is interleaved into spine group 0 (its w1tt/b1
    loads go right after ahi so ACT starts at ~4us, not 14); conv2 runs as
    8-row quarters, two at g==4 and two after the last group.  The
    s_group transpose stages through the cvb bank, not conv2's c2a.
  - conv1 / conv2 / G' matmuls run in bf16 (value paths; ~2.9e-3 rel err,
    well under the 2e-2 gate).  The G'->transpose->gather->conv1-TA chain
    stays fp32 (transposes must dtype-match their data, and a 16-bit
    transpose output cannot accumulate into an fp32 PSUM group).
  - PSUM (8 banks): spine 2x[128,1024] pairs (4) + conv1 cva/cvb (2) +
    conv2/transpose-staging c2a/c2b (2).  Sharing one bank between two
    64-partition accumulation groups passes the simulator but breaks on
    hardware (start=True zeroes the whole region) - keep halves in
    separate banks.
"""

import numpy as np
import ml_dtypes

import concourse.bass as bass
import concourse.bacc as bacc
import concourse.mybir as mybir
from concourse.tile import TileContext
from concourse import bass_utils
from concourse.masks import make_identity

F32 = mybir.dt.float32
BF16 = mybir.dt.bfloat16
I32 = mybir.dt.int32
AF = mybir.ActivationFunctionType
OP = mybir.AluOpType

B, C, W, H = 4, 64, 64, 64
HW = W * H                     # 4096
BN_EPS = 1e-5
N_CORES = 8
WROWS = W // 2 + 2             # 34 window rows (half + 1-row halo each side)
JW = WROWS * H                 # 2176 j-columns per core
JT = JW // 128                 # 17 j-tiles
NCH = HW // 512                # 8 i-chunks
OUT_ROWS = W // 2              # 32 interior rows per core
YPAD = H + 2                   # 66 padded y positions in fused layout

NEG = -3.0e38


def _build_nc():
    nc = bacc.Bacc("TRN2", target_bir_lowering=False)

    # Exact bf16 split pairs (X = X1 + X2 with X2 itself bf16-exact to
    # ~2^-17 rel).  T = V1'A1 + (V1'A2 + V2'A1): the two cross terms are
    # computed as ONE K=128 matmul by stacking [V1;V2] against [A2;A1] -
    # PE cost is per output row, independent of K.
    #   ahi: rows 0:64 = A1_q, rows 64:128 = A1_k
    #   axq/axk: rows 0:64 = A2_m, rows 64:128 = A1_m  (match m)
    #   vhi: V1 duplicated on both row halves; vx: rows 0:64 V1, 64:128 V2
    ahi = nc.dram_tensor("ahi", [128, HW], BF16, kind="ExternalInput")
    axq = nc.dram_tensor("axq", [128, HW], BF16, kind="ExternalInput")
    axk = nc.dram_tensor("axk", [128, HW], BF16, kind="ExternalInput")
    vhi = nc.dram_tensor("vhi", [128, JW], BF16, kind="ExternalInput")
    vx = nc.dram_tensor("vx", [128, JW], BF16, kind="ExternalInput")
    vwin = nc.dram_tensor("vwin", [C, JW], BF16, kind="ExternalInput")
    w1vt = nc.dram_tensor("w1vt", [C, C], BF16, kind="ExternalInput")
    w1tt = nc.dram_tensor("w1tt", [128, C], BF16, kind="ExternalInput")
    b1d = nc.dram_tensor("b1d", [C, 1], F32, kind="ExternalInput")
    w2ad = nc.dram_tensor("w2ad", [128, 9 * C], BF16, kind="ExternalInput")
    w2bd = nc.dram_tensor("w2bd", [C, 9 * C], BF16, kind="ExternalInput")
    bnad = nc.dram_tensor("bnad", [128, 1], F32, kind="ExternalInput")
    bnbd = nc.dram_tensor("bnbd", [128, 1], F32, kind="ExternalInput")
    yout = nc.dram_tensor("y", [C, OUT_ROWS * H], F32, kind="ExternalOutput")

    if True:
      with TileContext(nc) as tc:
        with tc.tile_pool(name="persist", bufs=1) as pp:
            ahi_t = pp.tile([128, HW], BF16)
            axq_t = pp.tile([128, HW], BF16)
            axk_t = pp.tile([128, HW], BF16)
            vhi_t = pp.tile([128, JW], BF16)
            vx_t = pp.tile([128, JW], BF16)
            w1vt_t = pp.tile([C, C], BF16)
            w1tt_t = pp.tile([128, C], BF16)
            b1_t = pp.tile([C, 1], F32)
            w2a_t = pp.tile([128, 9 * C], BF16)
            w2b_t = pp.tile([C, 9 * C], BF16)
            bna_t = pp.tile([128, 1], F32)
            bnb_t = pp.tile([128, 1], F32)
            iota_t = pp.tile([128, HW], F32)
            ident = pp.tile([128, 128], F32)
            gtile = [pp.tile([128, JT * C], F32, tag="gtq", name="gtq_t"),
                     pp.tile([128, JT * C], F32, tag="gtk", name="gtk_t")]
            s_all = [pp.tile([128, JT], F32, tag="sq", name="sq_t"),
                     pp.tile([128, JT], F32, tag="sk", name="sk_t")]
            idx_all = [pp.tile([128, JT], I32, tag="idxq", name="idxq_t"),
                       pp.tile([128, JT], I32, tag="idxk", name="idxk_t")]
            s_bc = pp.tile([128, JW], F32)     # rows 0:64 Sq, 64:128 Sk
            fused_a = pp.tile([128, WROWS * YPAD], BF16)  # C_v / C_k
            fused_b = pp.tile([C, WROWS * YPAD], BF16)    # V, y-padded
            out_sb = pp.tile([128, OUT_ROWS * H // 2], F32)

            # load order: G' work needs only ahi+w1tt+b1 (starts ACT
            # early); the first spine tile then needs vhi/axq/vx
            nc.sync.dma_start(out=ahi_t[:], in_=ahi[:])
            nc.sync.dma_start(out=w1tt_t[:], in_=w1tt[:])
            nc.sync.dma_start(out=b1_t[:], in_=b1d[:])
            nc.sync.dma_start(out=vhi_t[:], in_=vhi[:])
            nc.sync.dma_start(out=axq_t[:], in_=axq[:])
            nc.sync.dma_start(out=vx_t[:], in_=vx[:])
            nc.sync.dma_start(out=axk_t[:], in_=axk[:])
            nc.sync.dma_start(out=w1vt_t[:], in_=w1vt[:])
            nc.sync.dma_start(out=w2a_t[:], in_=w2ad[:])
            nc.sync.dma_start(out=w2b_t[:], in_=w2bd[:])
            nc.sync.dma_start(out=bna_t[:], in_=bnad[:])
            nc.sync.dma_start(out=bnb_t[:], in_=bnbd[:])
            nc.gpsimd.iota(iota_t[:], pattern=[[1, HW]], base=0,
                           channel_multiplier=0,
                           allow_small_or_imprecise_dtypes=True)
            make_identity(nc, ident[:])

            fb3 = fused_b[:].rearrange("c (x y) -> c x y", y=YPAD)
            nc.gpsimd.memset(fused_b[:], 0.0)
            nc.sync.dma_start(
                out=fb3[:, :, 1:H + 1],
                in_=vwin[:].rearrange("c (x y) -> c x y", y=H))
            nc.gpsimd.memset(gtile[0][:], 0.0)
            nc.gpsimd.memset(gtile[1][:], 0.0)

            with tc.tile_pool(name="gdram", bufs=1, space="DRAM") as gdr:
                gt_dram = [gdr.tile([HW, C], F32, tag="gtdq", name="gtdq_t"),
                           gdr.tile([HW, C], F32, tag="gtdk", name="gtdk_t")]

                # ---- Phases 3-6 interleaved ----
                fa3 = fused_a[:].rearrange("c (x y) -> c x y", y=YPAD)
                nc.gpsimd.memset(fa3[:, :, 0:1], 0.0)
                nc.gpsimd.memset(fa3[:, :, YPAD - 1:YPAD], 0.0)
                with tc.tile_pool(name="sp_ps", bufs=1, space="PSUM") as sps, \
                     tc.tile_pool(name="sp_sb", bufs=2) as ssb, \
                     tc.tile_pool(name="sp_sm", bufs=4) as ssm, \
                     tc.tile_pool(name="cv_ps", bufs=1, space="PSUM") as cvp, \
                     tc.tile_pool(name="s4_sb", bufs=2) as s4, \
                     tc.tile_pool(name="s4_dram", bufs=1, space="DRAM") as d4:

                    # ---- Phase 1+2: G' = W1t @ A1 + b1; transpose to DRAM.
                    # Emitted as closures interleaved into spine group 0 so
                    # the startup isn't serial (spine doesn't depend on G').
                    g_sbs = [pp.tile([C, HW], F32, tag="gsbq", name="gsbq"),
                             pp.tile([C, HW], F32, tag="gsbk", name="gsbk")]

                    def g_chunk(c8):
                        pms = [cvp.tile([C, 512], F32, tag="cva", name="gmq"),
                               cvp.tile([C, 512], F32, tag="cvb", name="gmk")]
                        for m in range(2):
                            ro = m * C
                            nc.tensor.matmul(
                                pms[m][:], w1tt_t[ro:ro + C, :],
                                ahi_t[ro:ro + C, c8 * 512:(c8 + 1) * 512],
                                start=True, stop=True,
                                tile_position=(ro, 0))
                        for m in range(2):
                            nc.scalar.activation(
                                g_sbs[m][:, c8 * 512:(c8 + 1) * 512],
                                pms[m][:],
                                AF.Identity, bias=b1_t[:, 0:1], scale=1.0)

                    def g_transpose(m, grp):
                        g_sb = g_sbs[m]
                        pst = cvp.tile([128, 512], F32, tag="c2a",
                                       name="gtr")
                        stg = pp.tile([128, 512], F32, tag="stg")
                        for t in range(8):
                            blk = grp * 8 + t
                            nc.tensor.matmul(
                                pst[:, t * C:(t + 1) * C],
                                g_sb[:, blk * 128:(blk + 1) * 128],
                                ident[0:C, 0:C], is_transpose=True,
                                start=True, stop=True)
                        nc.scalar.copy(stg[:], pst[:])
                        nc.sync.dma_start(
                            out=gt_dram[m][:]
                            .rearrange("(g p) c -> p g c", p=128)
                            [:, grp * 8:(grp + 1) * 8, :],
                            in_=stg[:].rearrange("p (g c) -> p g c", c=C))

                    g_work = [lambda c8=c8: g_chunk(c8) for c8 in range(NCH)]
                    g_work += [lambda m=m, grp=grp: g_transpose(m, grp)
                               for m in range(2) for grp in range(4)]

                    pend_gather = []
                    pend_count = []
                    neg1_t = pp.tile([128, 1], F32, tag="neg1",
                                     name="neg1")
                    nc.gpsimd.memset(neg1_t[:], -1.0)

                    def spine_mjt(m, jt, use_act):
                        # One match x one 128-j tile: 8 i-chunk matmul
                        # triples into 2-bank PSUM pair tiles, ACT evacuates
                        # [128,1024] pairs into T_sb, DVE does the 2x max
                        # pass, and the 1x is_equal*iota index pass runs on
                        # DVE or Pool (gpsimd) per the balance schedule.
                        tsb = ssb.tile([128, HW], F32, tag=f"tsb{m}",
                                       name=f"tsb{m}", bufs=3 if m == 0 else 2)
                        ro = m * C
                        js = slice(jt * 128, (jt + 1) * 128)
                        ax_t = axq_t if m == 0 else axk_t
                        for pr in range(4):
                            ps = sps.tile([128, 1024], F32,
                                          tag=f"sp{pr % 2}", name="sp")
                            for h in range(2):
                                ch = pr * 2 + h
                                cs = slice(ch * 512, (ch + 1) * 512)
                                o = slice(h * 512, (h + 1) * 512)
                                nc.tensor.matmul(
                                    ps[:, o], vhi_t[ro:ro + C, js],
                                    ahi_t[ro:ro + C, cs],
                                    start=True, stop=False,
                                    tile_position=(ro, 0))
                                nc.tensor.matmul(
                                    ps[:, o], vx_t[:, js],
                                    ax_t[:, cs],
                                    start=False, stop=True)
                            nc.scalar.copy(
                                tsb[:, pr * 1024:(pr + 1) * 1024], ps[:])
                        S = s_all[m][:, jt:jt + 1]
                        if use_act:
                            # Offload the index pass to ScalarE: P =
                            # prefix-max(T) (DVE scan, in place), then
                            # idx = -sum(sign(P - S)) via one ACT Sign
                            # activation with per-partition bias -S and the
                            # sum accumulator.  Exact, first-occurrence.
                            # The ACT part is emitted one match-tile later
                            # (via pend_count) so ACT's in-order queue never
                            # waits on the scan.
                            nc.vector.tensor_tensor_scan(
                                out=tsb[:], data0=tsb[:], data1=tsb[:],
                                initial=NEG, op0=OP.max, op1=OP.bypass)
                            nia = ssm.tile([128, 1], F32, tag="nia",
                                           name="nia")
                            nc.gpsimd.tensor_copy(
                                S, tsb[:, HW - 1:HW])
                            # idx = sum(sign(S - P)): scale=-1 with bias=S
                            # read straight from the scan's last column, so
                            # the accumulated count is directly positive.
                            nc.scalar.activation(
                                tsb[:], tsb[:], AF.Sign,
                                bias=tsb[:, HW - 1:HW], scale=-1.0,
                                accum_out=nia[:])
                            nc.scalar.activation(
                                idx_all[m][:, jt:jt + 1], nia[:],
                                AF.Copy, bias=0.0, scale=1.0)
                        else:
                            nc.vector.tensor_scalar(
                                out=tsb[:], in0=tsb[:],
                                scalar1=NEG, scalar2=NEG,
                                op0=OP.max, op1=OP.max, accum_out=S)
                            ist = ssm.tile([128, 1], F32, tag="ist",
                                           name="ist")
                            nc.vector.scalar_tensor_tensor(
                                out=tsb[:], in0=tsb[:], scalar=S,
                                in1=iota_t[:], op0=OP.is_equal,
                                op1=OP.mult, accum_out=ist[:])
                            nc.vector.tensor_copy(
                                idx_all[m][:, jt:jt + 1], ist[:])

                        def fire(m=m, jt=jt):
                            nc.gpsimd.indirect_dma_start(
                                out=gtile[m][:, jt * C:(jt + 1) * C],
                                out_offset=None,
                                in_=gt_dram[m][:],
                                in_offset=bass.IndirectOffsetOnAxis(
                                    ap=idx_all[m][:, jt:jt + 1], axis=0),
                                bounds_check=HW - 1, oob_is_err=False)
                        pend_gather.append(fire)

                    s_dram = [d4.tile([JW], F32, tag="sdq", name="sdq"),
                              d4.tile([JW], F32, tag="sdk", name="sdk")]

                    def s_group(g, jts):
                        n0 = jts[0] * 128
                        n1 = (jts[-1] + 1) * 128
                        nt = len(jts)
                        for m in range(2):
                            pst = cvp.tile([nt, 128], F32, tag="cvb",
                                           name="pst")
                            nc.tensor.matmul(
                                pst[:], s_all[m][:, jts[0]:jts[-1] + 1],
                                ident[:], is_transpose=True,
                                start=True, stop=True)
                            stg = s4.tile([JT, 128], F32, tag="stg4",
                                          name="stg4")
                            nc.scalar.copy(stg[0:nt, :], pst[:])
                            nc.sync.dma_start(
                                out=s_dram[m][n0:n1]
                                .rearrange("(t p) -> t p", p=128),
                                in_=stg[0:nt, :])
                            nc.sync.dma_start(
                                out=s_bc[m * C:(m + 1) * C, n0:n1],
                                in_=s_dram[m][None, n0:n1]
                                .to_broadcast((C, n1 - n0)))

                    def conv1_chunk(cn):
                        jts = list(range(4 * cn, min(4 * cn + 4, JT)))
                        n0 = cn * 512
                        n1 = min(n0 + 512, JW)
                        psm = [cvp.tile([128, 512], F32, tag="cva",
                                        name="cva"),
                               cvp.tile([128, 512], F32, tag="cvb",
                                        name="cvb")]
                        for m in range(2):
                            nc.tensor.matmul(
                                psm[m][m * C:(m + 1) * C, 0:n1 - n0],
                                w1vt_t[:], vhi_t[0:C, n0:n1],
                                start=True, stop=False,
                                tile_position=(0, m * C))
                        for m in range(2):
                            for i, jt in enumerate(jts):
                                if m == 0:
                                    nc.tensor.matmul(
                                        psm[m][0:C, i * 128:(i + 1) * 128],
                                        gtile[m][:, jt * C:(jt + 1) * C],
                                        ident[:], is_transpose=True,
                                        start=False, stop=(jt == jts[-1]))
                                else:
                                    nc.tensor.matmul(
                                        psm[m][C:128,
                                               i * 128:(i + 1) * 128],
                                        gtile[m][:, jt * C:(jt + 1) * C],
                                        ident[:],
                                        start=False, stop=(jt == jts[-1]),
                                        tile_position=(0, C))
                        x0 = n0 // H
                        nx = (n1 - n0) // H
                        for m in range(2):
                            nc.vector.tensor_tensor(
                                out=fa3[m * C:(m + 1) * C,
                                        x0:x0 + nx, 1:H + 1],
                                in0=psm[m][m * C:(m + 1) * C, 0:n1 - n0],
                                in1=s_bc[m * C:(m + 1) * C, n0:n1],
                                op=OP.mult)

                    def conv2_q(q):
                        # one 8-row quarter of the output (needs conv1
                        # chunks q and q+1 for its fused-row window)
                        half = q % 2
                        co = slice(half * C, (half + 1) * C)
                        psm = cvp.tile([128, 512], F32,
                                       tag=("c2a" if half == 0 else "c2b"),
                                       name="c2q")
                        ox = 1 + q * 8
                        for t in range(9):
                            dx, dy = t // 3, t % 3
                            ra = fa3[:, ox + dx - 1:ox + dx + 7,
                                     dy:dy + H]
                            rb = fb3[:, ox + dx - 1:ox + dx + 7,
                                     dy:dy + H]
                            nc.tensor.matmul(
                                psm[co, :],
                                w2a_t[:, t * C:(t + 1) * C], ra,
                                start=(t == 0), stop=False,
                                tile_position=(0, half * C))
                            nc.tensor.matmul(
                                psm[co, :],
                                w2b_t[:, t * C:(t + 1) * C], rb,
                                start=False, stop=(t == 8),
                                tile_position=(0, half * C))
                        ob = slice((q // 2) * 512, (q // 2) * 512 + 512)
                        nc.scalar.activation(
                            out_sb[co, ob],
                            psm[co, :], AF.Relu,
                            bias=bnb_t[co, 0:1], scale=bna_t[co, 0:1])
                        y3 = yout[:].rearrange("c (x y) -> c x y", y=H)
                        nc.sync.dma_start(
                            out=y3[:, q * 8:q * 8 + 8, :],
                            in_=out_sb[co, ob]
                            .rearrange("c (x y) -> c x y", y=H))

                    # Software-pipelined emission: group g's s_group/conv1
                    # are emitted after group g+1's spine, so their data deps
                    # (gathers, S transposes) are a full group stale and
                    # never head-block the in-order PE/Pool queues.  Gathers
                    # are deferred 2 match-tiles for the same reason.
                    def flush_gathers(keep=0):
                        while len(pend_gather) > keep:
                            pend_gather.pop(0)()

                    def flush_counts(keep=0):
                        while len(pend_count) > keep:
                            pend_count.pop(0)()

                    groups = [list(range(4 * g, min(4 * g + 4, JT)))
                              for g in range(5)]
                    for g in range(5):
                        for jt in groups[g]:
                            for m in range(2):
                                # two units of G' work interleaved per
                                # match-tile keeps startup non-serial
                                for _ in range(2):
                                    if g_work:
                                        g_work.pop(0)()
                                i2 = 2 * jt + m
                                use_act = ((i2 + 1) * 12 // 34) > \
                                    (i2 * 12 // 34)
                                spine_mjt(m, jt, use_act)
                                flush_counts(keep=1)
                                # gt_dram is complete after group 0; hold
                                # gathers until then so Pool never stalls
                                flush_gathers(keep=2 if g >= 1 else 99)
                        if g >= 1:
                            s_group(g - 1, groups[g - 1])
                            conv1_chunk(g - 1)
                        if g == 4:
                            conv2_q(0)
                            conv2_q(1)
                    flush_counts(keep=0)
                    flush_gathers(keep=0)
                    s_group(4, groups[4])
                    conv1_chunk(4)
                    conv2_q(2)
                    conv2_q(3)

    nc.finalize()
    return nc


_NC_CACHE = None


def _get_nc():
    global _NC_CACHE
    if _NC_CACHE is None:
        _NC_CACHE = _build_nc()
    return _NC_CACHE


def _bf16_split(x):
    hi = x.astype(ml_dtypes.bfloat16)
    lo = (x - hi.astype(np.float32)).astype(ml_dtypes.bfloat16)
    return hi, lo


def _host_prep(inputs):
    V = np.ascontiguousarray(inputs["V_rgb"], dtype=np.float32)
    K = np.ascontiguousarray(inputs["K_dep"], dtype=np.float32)
    Q = np.ascontiguousarray(inputs["Q_flo"], dtype=np.float32)
    w1 = np.asarray(inputs["conv1_w"], dtype=np.float32)[:, :, 0, 0]
    b1 = np.asarray(inputs["conv1_b"], dtype=np.float32)
    w2 = np.asarray(inputs["conv2_w"], dtype=np.float32)
    b2 = np.asarray(inputs["conv2_b"], dtype=np.float32)
    g = np.asarray(inputs["bn_gamma"], dtype=np.float32)
    be = np.asarray(inputs["bn_beta"], dtype=np.float32)
    mu = np.asarray(inputs["bn_mean"], dtype=np.float32)
    var = np.asarray(inputs["bn_var"], dtype=np.float32)

    w1vt = np.ascontiguousarray(w1[:, :C].T).astype(ml_dtypes.bfloat16)
    w1tt1 = np.ascontiguousarray(w1[:, C:].T)
    w1tt = np.concatenate([w1tt1, w1tt1], axis=0).astype(ml_dtypes.bfloat16)
    w2a = np.zeros((128, 9 * C), np.float32)
    w2b = np.zeros((C, 9 * C), np.float32)
    for t in range(9):
        dx, dy = t // 3, t % 3
        lhsT = w2[:, :, dx, dy].T                     # [192, 64]
        w2a[:, t * C:(t + 1) * C] = lhsT[0:128]
        w2b[:, t * C:(t + 1) * C] = lhsT[128:192]
    w2a = w2a.astype(ml_dtypes.bfloat16)
    w2b = w2b.astype(ml_dtypes.bfloat16)
    bna = g / np.sqrt(var + BN_EPS)
    bnb = be + (b2 - mu) * bna
    bna2 = np.ascontiguousarray(np.concatenate([bna, bna])[:, None])
    bnb2 = np.ascontiguousarray(np.concatenate([bnb, bnb])[:, None])

    in_maps = []
    for core in range(N_CORES):
        b, half = core // 2, core % 2
        x0 = half * (W // 2)
        vw = np.zeros((C, WROWS, H), np.float32)
        lo = x0 - 1
        hi = x0 + W // 2 + 1
        slo, shi = max(lo, 0), min(hi, W)
        vw[:, slo - lo:slo - lo + (shi - slo), :] = V[b, :, slo:shi, :]
        vw = vw.reshape(C, JW)
        aq = Q[b].reshape(C, HW)
        ak = K[b].reshape(C, HW)
        aq1, aq2 = _bf16_split(aq)
        ak1, ak2 = _bf16_split(ak)
        v1, v2 = _bf16_split(vw)
        ahi = np.concatenate([aq1, ak1], axis=0)      # [128, HW]
        axq = np.concatenate([aq2, aq1], axis=0)
        axk = np.concatenate([ak2, ak1], axis=0)
        vhi = np.concatenate([v1, v1], axis=0)        # [128, JW]
        vx = np.concatenate([v1, v2], axis=0)
        in_maps.append({
            "ahi": np.ascontiguousarray(ahi),
            "axq": np.ascontiguousarray(axq),
            "axk": np.ascontiguousarray(axk),
            "vhi": np.ascontiguousarray(vhi),
            "vx": np.ascontiguousarray(vx),
            "vwin": np.ascontiguousarray(vw.astype(ml_dtypes.bfloat16)),
            "w1vt": w1vt,
            "w1tt": w1tt,
            "b1d": np.ascontiguousarray(b1[:, None]),
            "w2ad": w2a,
            "w2bd": w2b,
            "bnad": bna2,
            "bnbd": bnb2,
        })
    return in_maps


def kernel(**inputs):
    nc = _get_nc()
    in_maps = _host_prep(inputs)
    res = bass_utils.run_bass_kernel_spmd(
        nc, in_maps, core_ids=list(range(N_CORES)))
    y = np.zeros((B, C, W, H), np.float32)
    for core in range(N_CORES):
        b, half = core // 2, core % 2
        x0 = half * (W // 2)
        y[b, :, x0:x0 + W // 2, :] = \
            res.results[core]["y"].reshape(C, OUT_ROWS, H)
    return y
